# revision 15
# baseline (speedup 1.0000x reference)
"""Trainium2 Bass kernel for a dense transformer block (pre-LN attention + MLP).

Sharding: 8 cores, pure data/sequence parallel, zero collectives.
Core c handles batch b=c//2 and query-half h=c%2 (1024 query tokens).
Each core redundantly computes K/V for its full batch (2048 tokens), which is
cheaper than a cross-core KV exchange on this chip.  The per-core x shard is
rolled so the core's own 1024 query tokens are always rows 0:1024 (attention
here is permutation-invariant over keys, so rolling keys is harmless).

Host-side folding (numpy):
  ln1 affine -> qkv weights/bias;  1/sqrt(dh) -> q weights/bias
  ls1 -> proj weights/bias;  ln2 affine -> fc1;  ls2 -> fc2
so the device only computes raw (affine-free) layernorms and plain matmuls.
Weights are pre-scaled by powers of two into fp8 e4m3's normal range; the
inverse scale is folded into each PSUM eviction (free on ACT/DVE affine ops).

Device dataflow (fp8 DoubleRow matmuls + f32 residual spine):
  LN1 (normalize split ACT/DVE) -> PE-transpose -> qT/kT computed feature-major
  bf16, V token-major fp8 with a ones column per 65-wide head block (softmax
  denominators fall out of the AV matmul for free); scores computed transposed
  [k, q] in bf16 with the PE array row-tiled 64x128: even head on tile (0,0),
  odd head on tile (64,0), interleaved so both halves stream concurrently.
  Exp split across ACT (table exp) and DVE (fp8-bit-space affine trick).
  AV in fp8 DoubleRow; denominator row evicted with the AV values to SBUF by
  ACT, reciprocal + broadcast (DRAM hop) + multiply on DVE in all-SBUF mode.
  All contraction-256 matmuls (QKV, V, AV, proj, fc1, fc2) run fp8 DoubleRow.
"""

import sys

sys.path.insert(0, "/opt/trn_rl_repo")

from contextlib import ExitStack

import numpy as np
import ml_dtypes

import concourse.bass as bass  # noqa: F401
import concourse.tile as tile
from concourse import bacc, mybir
from concourse.bass_utils import run_bass_kernel_spmd

B, N, D = 4, 2048, 768
H, DH = 12, 64
HID = 4 * D
EPS = 1e-5
P = 128
TKV = 2048  # tokens per core for K/V (full batch)
TQ = 1024  # query tokens per core
NT_KV = TKV // P  # 16
NT_Q = TQ // P  # 8
ND = D // P  # 6
NH = HID // P  # 24
HW = DH + 1  # head width in v_sb (64 V cols + ones col)
VW = 784  # v_sb row width: 12*65=780 padded to %16 for DoubleRow
F32 = mybir.dt.float32
BF16 = mybir.dt.bfloat16
F8 = mybir.dt.float8e4
F8NP = ml_dtypes.float8_e4m3
OP = mybir.AluOpType
ACTF = mybir.ActivationFunctionType
DR = mybir.MatmulPerfMode.DoubleRow
GELU_FUNC = ACTF.Gelu  # test_sim swaps to Identity (CoreSim lacks Gelu)

# power-of-two weight prescales (into fp8 normal range), descaled on eviction
S_QKV = 2.0 ** 6
S_PROJ = 2.0 ** 22
S_FC1 = 2.0 ** 6
S_FC2 = 2.0 ** 22

# fp8-bit-space exp approximation (DVE half of the exp work):
#   e4m3_bits(exp(x)) ~= trunc(SCHRA*x + SCHRB) for x in [-4.8, +3.9]
# scores are N(0, ~0.55) so the affine never goes negative/overflows.
SCHRA = 8.0 / float(np.log(2.0))
SCHRB = 56.04  # trunc-calibrated (CoreSim/HW convert truncates)
# exp engine split: 1 = ACT table exp, 0 = DVE bit trick, cycled per chunk
EXPMASK = (1, 0, 0, 1, 0, 0, 1, 0, 0, 1, 0, 0, 1, 0, 0, 1)
# extra power-of-two scale on q and k weights so fp8 q/k land mid-range;
# scores come out x64 and the exp evictions descale by SC_EXP.
QK_EXTRA = 8.0
SC_EXP = 1.0 / (QK_EXTRA * QK_EXTRA)


def _ln_stats(nc, pool, x_tile, eps_t):
    """mean, rstd of a [128, 768] f32 tile over free dim.

    Sum(x) on DVE (reduce), Sum(x^2) on ACT (Square + accum_out, output to a
    scratch tile) so the big free-dim passes split across both engines.
    """
    v = nc.vector
    sx = pool.tile([P, 1], F32, tag="sx")
    v.reduce_sum(sx[:, :], x_tile, axis=mybir.AxisListType.X)
    scr = pool.tile([P, D], F32, tag="scr")
    sxx = pool.tile([P, 1], F32, tag="sxx")
    nc.scalar.activation(scr[:, :], x_tile, ACTF.Square, accum_out=sxx[:, :])
    mu = pool.tile([P, 1], F32, tag="mu")
    nc.scalar.mul(mu[:, :], sx[:, :], 1.0 / D)
    musq = pool.tile([P, 1], F32, tag="musq")
    v.tensor_mul(musq[:, :], mu[:, :], mu[:, :])
    rs = pool.tile([P, 1], F32, tag="rs")
    # var + eps = sxx/D - mu^2 + eps; eps folded via the Sqrt bias
    v.scalar_tensor_tensor(rs[:, :], sxx[:, :], 1.0 / D, musq[:, :],
                           op0=OP.mult, op1=OP.subtract)
    nc.scalar.activation(rs[:, :], rs[:, :], ACTF.Sqrt, bias=eps_t[:, :])
    v.reciprocal(rs[:, :], rs[:, :])
    return mu, rs


def _ln_transpose(nc, tc, pools, src_tiles, nt, dst, eps_t, ident, tag):
    """LN (no affine) each [128, 768] f32 tile of src, transpose into dst
    [P, ND, nt*128] fp8.  Normalize alternates ACT/DVE per tile."""
    v = nc.vector
    stat_pool, lnp, tps = pools
    for ti in range(nt):
        xt = src_tiles(ti)
        mu, rs = _ln_stats(nc, stat_pool, xt, eps_t)
        xn = lnp.tile([P, D], BF16, tag=f"xn{tag}")
        if ti % 2 == 0:
            # ACT: Identity(x*rs + (-mu*rs)) with per-partition scale/bias
            nmr = stat_pool.tile([P, 1], F32, tag="nmr")
            v.scalar_tensor_tensor(nmr[:, :], mu[:, :], -1.0, rs[:, :],
                                   op0=OP.mult, op1=OP.mult)
            nc.scalar.activation(xn[:, :], xt, ACTF.Identity,
                                 bias=nmr[:, :], scale=rs[:, :])
        else:
            v.tensor_scalar(xn[:, :], xt, mu[:, :], rs[:, :],
                            op0=OP.subtract, op1=OP.mult)
        for dj in range(ND):
            pst = tps.tile([P, P], BF16, tag=f"t{tag}")
            nc.tensor.transpose(pst[:, :], xn[:, dj * P:(dj + 1) * P], ident[:, :])
            if (ti * ND + dj) % 2 == 0:
                nc.scalar.copy(dst[:, dj, ti * P:(ti + 1) * P], pst[:, :])
            else:
                v.tensor_copy(dst[:, dj, ti * P:(ti + 1) * P], pst[:, :])


def build_graph(repeat=1):
    nc = bacc.Bacc("TRN2", target_bir_lowering=False, debug=False, num_devices=8)

    x_ext = nc.declare_dram_parameter("x", [TKV, D], F32, isOutput=False)
    wqkv_ext = nc.declare_dram_parameter("wqkv", [D, 3 * D], F8, isOutput=False)
    wproj_ext = nc.declare_dram_parameter("wproj", [D, D], F8, isOutput=False)
    w1_ext = nc.declare_dram_parameter("w1", [D, HID], F8, isOutput=False)
    w2_ext = nc.declare_dram_parameter("w2", [HID, D], F8, isOutput=False)
    bqkv_ext = nc.declare_dram_parameter("bqkv", [P, 12], F32, isOutput=False)
    b1_ext = nc.declare_dram_parameter("b1", [P, NH], F32, isOutput=False)
    ident_ext = nc.declare_dram_parameter("ident", [P, P], BF16, isOutput=False)
    out_ext = nc.declare_dram_parameter("out", [TQ, D], F32, isOutput=True)

    with tile.TileContext(nc) as tc:
        for _ in range(repeat):
            emit(nc, tc, x_ext.ap(), out_ext.ap(), wqkv_ext.ap(), wproj_ext.ap(),
                 w1_ext.ap(), w2_ext.ap(), bqkv_ext.ap(), b1_ext.ap(),
                 ident_ext.ap())

    nc.compile()
    return nc


def emit(nc, tc, x, out, wqkv_d, wproj_d, w1_d, w2_d, bqkv_d, b1_d, ident_d):
    v = nc.vector
    sc = nc.scalar
    te = nc.tensor

    ctx = ExitStack()
    with ctx:
        # ---------- kernel-lifetime pools ----------
        singles = ctx.enter_context(tc.tile_pool(name="singles", bufs=1))
        stat_pool = ctx.enter_context(tc.tile_pool(name="stat", bufs=4))

        eps_t = singles.tile([P, 1], F32)
        v.memset(eps_t[:, :], EPS)
        ident = singles.tile([P, P], BF16)
        nc.sync.dma_start(ident[:, :], ident_d[:, :])
        bqkv = singles.tile([P, 12], F32)
        nc.sync.dma_start(bqkv[:, :], bqkv_d[:, :])
        b1c = singles.tile([P, NH], F32)
        nc.sync.dma_start(b1c[:, :], b1_d[:, :])

        resid = ctx.enter_context(tc.tile_pool(name="resid", bufs=1))
        x1 = resid.tile([P, NT_Q, D], F32)

        with ExitStack() as attn_ctx:
            xownp = attn_ctx.enter_context(tc.tile_pool(name="xownp", bufs=1))
            x_own = xownp.tile([P, NT_Q, D], F32)  # own tokens, residual spine
            qkvp = attn_ctx.enter_context(tc.tile_pool(name="qkvp", bufs=1))
            # q/k packed for fp8 DoubleRow scores: head h lives on partitions
            # [32*(h%3), 32*(h%3)+32) at free index j=h//3; head-dim d maps to
            # (partition 32*(h%3) + d%32, slot d//32).  Base partition is
            # hardware-limited to {0,32,64}, so only 3 of 4 groups are used.
            qTd = qkvp.tile([P, 4, 2, TQ], F8)
            kTd = qkvp.tile([P, 4, 2, TKV], F8)
            v_sb = qkvp.tile([P, NT_KV, VW], F8)
            wqkv = qkvp.tile([P, ND, 3 * D], F8)
            for dj in range(ND):
                nc.sync.dma_start(wqkv[:, dj, :], wqkv_d[dj * P:(dj + 1) * P, :])
            xnT = qkvp.tile([P, ND, TKV], F8)
            wproj = qkvp.tile([P, ND, D], F8)
            for dj in range(ND):
                nc.sync.dma_start(wproj[:, dj, :], wproj_d[dj * P:(dj + 1) * P, :])
            attnT = qkvp.tile([P, ND, TQ], F8)

            # ---- phase A: load x, LN1, transpose ----
            with tc.tile_pool(name="xkv", bufs=3) as xkvp, \
                 tc.tile_pool(name="ln1", bufs=4) as lnp, \
                 tc.tile_pool(name="tps1", bufs=6, space="PSUM") as tps:
                def src(ti):
                    if ti < NT_Q:
                        nc.sync.dma_start(x_own[:, ti, :],
                                          x[ti * P:(ti + 1) * P, :])
                        return x_own[:, ti, :]
                    t = xkvp.tile([P, D], F32, tag="xkv")
                    nc.sync.dma_start(t[:, :], x[ti * P:(ti + 1) * P, :])
                    return t[:, :]

                _ln_transpose(nc, tc, (stat_pool, lnp, tps), src, NT_KV,
                              xnT, eps_t, ident, "1")

            # ---- merged phase B+C: QKV matmuls interleaved with attention ----
            # PSUM budget: sps 4x[128,512] = 4 banks + avps 2x[65,1024] = 4.
            with tc.tile_pool(name="sps", bufs=4, space="PSUM") as sps, \
                 tc.tile_pool(name="avps", bufs=2, space="PSUM") as avps, \
                 tc.tile_pool(name="expp", bufs=9) as expp, \
                 tc.tile_pool(name="abuf", bufs=3) as abuf, \
                 tc.tile_pool(name="recd", bufs=3, space="DRAM") as recdp, \
                 tc.tile_pool(name="recp", bufs=3) as recp, \
                 tc.tile_pool(name="qkt", bufs=4) as qkt:
                # ones columns of v_sb (col 64 of each 65-wide head block)
                vg = v_sb[:, :, 0:H * HW].rearrange("p a (h c) -> p a h c", h=H)
                v.memset(vg[:, :, :, DH:DH + 1], 1.0)

                def v_unit(ti):
                    # two 512-wide psum chunks; second only 256 used
                    for c, (lo, ln_, h0, h1) in enumerate(
                            ((0, 512, 0, 8), (512, 256, 8, 12))):
                        ps = sps.tile([P, 512], F32, tag="s")
                        for dp in range(ND // 2):
                            te.matmul(
                                ps[:, 0:ln_],
                                xnT[:, 2 * dp:2 * dp + 2, ti * P:(ti + 1) * P],
                                wqkv[:, 2 * dp:2 * dp + 2,
                                     2 * D + lo:2 * D + lo + ln_],
                                start=(dp == 0), stop=(dp == ND // 2 - 1),
                                perf_mode=DR,
                            )
                        pg = ps[:, 0:ln_].rearrange("p (h c) -> p h c", h=h1 - h0)
                        sc.activation(vg[:, ti, h0:h1, 0:DH], pg[:, :, :],
                                      ACTF.Copy, scale=1.0 / S_QKV)

                def qk_unit(fj, th):
                    """produce qTd (fj<6) or kTd (fj>=6) for block fj%6, th
                    half; evict fp8 then DMA-remap into the DR-pair layout."""
                    is_q = fj < ND
                    base = fj if is_q else fj - ND
                    dstT = qTd if is_q else kTd
                    for c in range(2):
                        lo = c * 512
                        ps = sps.tile([P, 512], F32, tag="s")
                        for dp in range(ND // 2):
                            te.matmul(
                                ps[:, :],
                                wqkv[:, 2 * dp:2 * dp + 2, fj * P:(fj + 1) * P],
                                xnT[:, 2 * dp:2 * dp + 2,
                                    th * 1024 + lo:th * 1024 + lo + 512],
                                start=(dp == 0), stop=(dp == ND // 2 - 1),
                                perf_mode=DR,
                            )
                        tmp = qkt.tile([P, 512], F8, tag="qkt")
                        sc.activation(tmp[:, :], ps[:, :], ACTF.Identity,
                                      bias=bqkv[:, fj:fj + 1], scale=1.0 / S_QKV)
                        tok0 = (0 if is_q else th * 1024) + lo
                        for hh in range(2):
                            h = 2 * base + hh
                            g, j = h % 3, h // 3
                            for s in range(2):
                                nc.sync.dma_start(
                                    dstT[32 * g:32 * g + 32, j, s,
                                         tok0:tok0 + 512],
                                    tmp[64 * hh + 32 * s:64 * hh + 32 * s + 32,
                                        0:512])

                def attn_pair(fj):
                    """Both heads of feature block fj: scores on PE row tiles
                    (0,0)/(64,0) interleaved, exp split ACT/DVE, then AV +
                    denominator for each head."""
                    eAB = ([], [])
                    ec = [0]

                    def exp_evict(dst, src_ps):
                        use_act = EXPMASK[ec[0] % len(EXPMASK)]
                        ec[0] += 1
                        if use_act:
                            sc.activation(dst, src_ps, ACTF.Exp, scale=SC_EXP)
                        else:
                            eb = dst.bitcast(mybir.dt.uint8)
                            v.tensor_scalar(eb, src_ps, SCHRA * SC_EXP, SCHRB,
                                            op0=OP.mult, op1=OP.add)

                    for ktp in range(NT_KV // 2):
                        for hh in range(2):
                            eAB[hh].append(expp.tile([P, 2, TQ], F8,
                                                     tag=f"e{hh}",
                                                     name=f"ep{hh}"))
                        for k2 in range(2):
                            kt = 2 * ktp + k2
                            for c in range(2):
                                lo = c * 512
                                for hh in range(2):
                                    h = 2 * fj + hh
                                    g, j = h % 3, h // 3
                                    ps = sps.tile([P, 512], F32, tag="s")
                                    te.matmul(
                                        ps[:, :],
                                        kTd[32 * g:32 * g + 32, j, :,
                                            kt * P:(kt + 1) * P],
                                        qTd[32 * g:32 * g + 32, j, :,
                                            lo:lo + 512],
                                        start=True, stop=True, perf_mode=DR,
                                    )
                                    exp_evict(eAB[hh][ktp][:, k2, lo:lo + 512],
                                              ps[:, :])

                    for hh in range(2):
                        h = 2 * fj + hh
                        po = hh * DH
                        epairs = eAB[hh]
                        av = avps.tile([DH + 1, TQ], F32, tag="av")
                        for ktp in range(NT_KV // 2):
                            for c in range(2):
                                lo = c * 512
                                te.matmul(
                                    av[:, lo:lo + 512],
                                    v_sb[:, 2 * ktp:2 * ktp + 2,
                                         h * HW:(h + 1) * HW],
                                    epairs[ktp][:, :, lo:lo + 512],
                                    start=(ktp == 0),
                                    stop=(ktp == NT_KV // 2 - 1),
                                    perf_mode=DR,
                                )
                        # evict values+denominator row to SBUF in one ACT op
                        av_sb = abuf.tile([DH + 1, TQ], BF16, tag="avsb")
                        sc.activation(av_sb[:, :], av[:, :], ACTF.Copy)
                        # reciprocal of the denom row: round-trip through DRAM
                        # to fold [1,1024] onto 8 partitions (single-lane DVE
                        # reciprocal is ~30x slower), then broadcast back.
                        dr8 = recdp.tile([8, TQ // 8], BF16, tag="rd8")
                        nc.sync.dma_start(dr8[:, :], av_sb[DH:DH + 1, :])
                        r8 = recp.tile([8, TQ // 8], BF16, tag="r8")
                        nc.sync.dma_start(r8[:, :], dr8[:, :])
                        r8o = recp.tile([8, TQ // 8], BF16, tag="r8o")
                        with nc.allow_low_precision(
                                reason="softmax denom reciprocal; block "
                                       "output is ls-gamma-scaled"):
                            v.reciprocal(r8o[:, :], r8[:, :])
                        recd = recdp.tile([1, TQ], BF16, tag="rd")
                        nc.sync.dma_start(recd[:, :], r8o[:, :])
                        recb = recp.tile([DH, TQ], BF16, tag="rb")
                        nc.sync.dma_start(recb[:, :],
                                          recd[0:1, :].to_broadcast((DH, TQ)))
                        v.tensor_tensor(attnT[po:po + DH, fj, :],
                                        av_sb[0:DH, :], recb[:, :], op=OP.mult)

                for ti in range(NT_KV):
                    v_unit(ti)
                for fj in range(ND):
                    qk_unit(fj, 0)          # qT[fj]
                    qk_unit(ND + fj, 0)     # kT[fj] first half
                    qk_unit(ND + fj, 1)     # kT[fj] second half
                    attn_pair(fj)

            # ---- phase D: proj + residual (fp8 DR) ----
            with tc.tile_pool(name="pps", bufs=4, space="PSUM") as pps:
                for ti in range(NT_Q):
                    ps = pps.tile([P, D], F32, tag="p")
                    for lo, ln_ in ((0, 512), (512, 256)):
                        for dp in range(ND // 2):
                            te.matmul(
                                ps[:, lo:lo + ln_],
                                attnT[:, 2 * dp:2 * dp + 2, ti * P:(ti + 1) * P],
                                wproj[:, 2 * dp:2 * dp + 2, lo:lo + ln_],
                                start=(dp == 0), stop=(dp == ND // 2 - 1),
                                perf_mode=DR,
                            )
                    v.scalar_tensor_tensor(x1[:, ti, :], ps[:, :], 1.0 / S_PROJ,
                                           x_own[:, ti, :], op0=OP.mult, op1=OP.add)
        # attnT / wproj / qT / kT / v_sb / x_own freed here

        # ---- phase E/F: LN2 + MLP (fp8 DR) ----
        with ExitStack() as mlp_ctx:
            w12p = mlp_ctx.enter_context(tc.tile_pool(name="w12", bufs=1))
            w1 = w12p.tile([P, ND, HID], F8)
            for dj in range(ND):
                nc.sync.dma_start(w1[:, dj, :], w1_d[dj * P:(dj + 1) * P, :])
            w2 = w12p.tile([P, NH, D], F8)
            for fj in range(NH):
                nc.sync.dma_start(w2[:, fj, :], w2_d[fj * P:(fj + 1) * P, :])

            h1T = mlp_ctx.enter_context(
                tc.tile_pool(name="h1Tp", bufs=1)).tile([P, NH, TQ], F8)

            with ExitStack() as fc1_ctx:
                xn2T = fc1_ctx.enter_context(
                    tc.tile_pool(name="xn2Tp", bufs=1)).tile([P, ND, TQ], F8)
                with tc.tile_pool(name="ln2", bufs=3) as lnp2, \
                     tc.tile_pool(name="tps2", bufs=8, space="PSUM") as tps2:
                    _ln_transpose(nc, tc, (stat_pool, lnp2, tps2),
                                  lambda ti: x1[:, ti, :], NT_Q, xn2T, eps_t,
                                  ident, "2")

                mps = mlp_ctx.enter_context(
                    tc.tile_pool(name="mps", bufs=3, space="PSUM"))
                if True:
                    for fj in range(NH):
                        ps = mps.tile([P, TQ], F32, tag="m")
                        for c in range(2):
                            lo = c * 512
                            for dp in range(ND // 2):
                                te.matmul(
                                    ps[:, lo:lo + 512],
                                    w1[:, 2 * dp:2 * dp + 2, fj * P:(fj + 1) * P],
                                    xn2T[:, 2 * dp:2 * dp + 2, lo:lo + 512],
                                    start=(dp == 0), stop=(dp == ND // 2 - 1),
                                    perf_mode=DR,
                                )
                        sc.activation(h1T[:, fj, :], ps[:, :], GELU_FUNC,
                                      bias=b1c[:, fj:fj + 1], scale=1.0 / S_FC1)
            # xn2T freed

            with tc.tile_pool(name="outp", bufs=2) as outp:
                for ti in range(NT_Q):
                    ps = mps.tile([P, TQ], F32, tag="m")
                    for lo, ln_ in ((0, 512), (512, 256)):
                        for fp_ in range(NH // 2):
                            te.matmul(
                                ps[:, lo:lo + ln_],
                                h1T[:, 2 * fp_:2 * fp_ + 2, ti * P:(ti + 1) * P],
                                w2[:, 2 * fp_:2 * fp_ + 2, lo:lo + ln_],
                                start=(fp_ == 0), stop=(fp_ == NH // 2 - 1),
                                perf_mode=DR,
                            )
                    ot = outp.tile([P, D], F32, tag="ot")
                    v.scalar_tensor_tensor(ot[:, :], ps[:, 0:D], 1.0 / S_FC2,
                                           x1[:, ti, :], op0=OP.mult, op1=OP.add)
                    nc.sync.dma_start(out[ti * P:(ti + 1) * P, :], ot[:, :])


def _fold(inputs):
    """Fold LN affines, layer scales, and 1/sqrt(dh) into weights (host numpy)."""
    f = {k: np.asarray(v, dtype=np.float32) for k, v in inputs.items()}
    wqkv = (f["ln1_w"][:, None] * f["qkv_w"]).copy()
    bqkv = (f["qkv_b"] + f["ln1_b"] @ f["qkv_w"]).copy()
    scale = 1.0 / np.sqrt(DH)
    wqkv[:, :D] *= scale
    bqkv[:D] *= scale
    wproj = f["proj_w"] * f["ls1_g"][None, :]
    bproj = f["proj_b"] * f["ls1_g"]
    w1 = f["ln2_w"][:, None] * f["fc1_w"]
    b1 = f["fc1_b"] + f["ln2_b"] @ f["fc1_w"]
    w2 = f["fc2_w"] * f["ls2_g"][None, :]
    b2 = f["fc2_b"] * f["ls2_g"]
    assert np.all(bproj == 0.0) and np.all(b2 == 0.0), (
        "nonzero proj/fc2 bias path not implemented")
    assert np.all(bqkv[2 * D:] == 0.0), "nonzero v bias path not implemented"
    return wqkv, bqkv, wproj, w1, b1, w2


def make_in_maps(inputs):
    x = np.asarray(inputs["x"], dtype=np.float32)
    wqkv, bqkv, wproj, w1, b1, w2 = _fold(inputs)
    # extra q/k scaling so fp8 q/k activations land mid-range; descaled at exp
    wqkv[:, :2 * D] *= QK_EXTRA
    bqkv[:2 * D] *= QK_EXTRA
    common = {
        "wqkv": (wqkv * S_QKV).astype(F8NP),
        "wproj": (wproj * S_PROJ).astype(F8NP),
        "w1": (w1 * S_FC1).astype(F8NP),
        "w2": (w2 * S_FC2).astype(F8NP),
        "bqkv": bqkv[:2 * D].reshape(12, P).T.copy().astype(np.float32),
        "b1": b1.reshape(NH, P).T.copy().astype(np.float32),
        "ident": np.eye(P, dtype=ml_dtypes.bfloat16),
    }
    in_maps = []
    for c in range(8):
        b, h = c // 2, c % 2
        xb = np.roll(x[b], -h * TQ, axis=0)
        in_maps.append({"x": np.ascontiguousarray(xb), **common})
    return in_maps


_CACHE = {}
TRACE = False


def kernel(**inputs):
    in_maps = make_in_maps(inputs)
    if "nc" not in _CACHE:
        _CACHE["nc"] = build_graph()
    nc = _CACHE["nc"]

    res = run_bass_kernel_spmd(nc, in_maps, core_ids=list(range(8)), trace=TRACE)
    _CACHE["last_result"] = res

    outp = np.empty((B, N, D), dtype=np.float32)
    for c in range(8):
        b, h = c // 2, c % 2
        outp[b, h * TQ:(h + 1) * TQ, :] = res.results[c]["out"]
    return outp


# revision 16
# speedup vs baseline: 1.2621x; 1.2621x over previous
"""Trainium2 Bass kernel for a dense transformer block (pre-LN attention + MLP).

Sharding: 8 cores, pure data/sequence parallel, zero collectives.
Core c handles batch b=c//2 and query-half h=c%2 (1024 query tokens).
Each core redundantly computes K/V for its full batch (2048 tokens), which is
cheaper than a cross-core KV exchange on this chip.  The per-core x shard is
rolled so the core's own 1024 query tokens are always rows 0:1024 (attention
here is permutation-invariant over keys, so rolling keys is harmless).

Host-side folding (numpy):
  ln1 affine -> qkv weights/bias;  1/sqrt(dh) -> q weights/bias
  ls1 -> proj weights/bias;  ln2 affine -> fc1;  ls2 -> fc2
so the device only computes raw (affine-free) layernorms and plain matmuls.
Weights are pre-scaled by powers of two into fp8 e4m3's normal range; the
inverse scale is folded into each PSUM eviction (free on ACT/DVE affine ops).

Device dataflow (fp8 DoubleRow matmuls + f32 residual spine):
  LN1 (normalize split ACT/DVE) -> PE-transpose -> qT/kT computed feature-major
  bf16, V token-major fp8 with a ones column per 65-wide head block (softmax
  denominators fall out of the AV matmul for free); scores computed transposed
  [k, q] in bf16 with the PE array row-tiled 64x128: even head on tile (0,0),
  odd head on tile (64,0), interleaved so both halves stream concurrently.
  Exp split across ACT (table exp) and DVE (fp8-bit-space affine trick).
  AV in fp8 DoubleRow; denominator row evicted with the AV values to SBUF by
  ACT, reciprocal + broadcast (DRAM hop) + multiply on DVE in all-SBUF mode.
  All contraction-256 matmuls (QKV, V, AV, proj, fc1, fc2) run fp8 DoubleRow.
"""

import sys

sys.path.insert(0, "/opt/trn_rl_repo")

from contextlib import ExitStack

import numpy as np
import ml_dtypes

import concourse.bass as bass  # noqa: F401
import concourse.tile as tile
from concourse import bacc, mybir
from concourse.bass_utils import run_bass_kernel_spmd

B, N, D = 4, 2048, 768
H, DH = 12, 64
HID = 4 * D
EPS = 1e-5
P = 128
TKV = 2048  # tokens per core for K/V (full batch)
TQ = 1024  # query tokens per core
NT_KV = TKV // P  # 16
NT_Q = TQ // P  # 8
ND = D // P  # 6
NH = HID // P  # 24
HW = DH + 1  # head width in v_sb (64 V cols + ones col)
VW = 784  # v_sb row width: 12*65=780 padded to %16 for DoubleRow
F32 = mybir.dt.float32
BF16 = mybir.dt.bfloat16
F8 = mybir.dt.float8e4
F8NP = ml_dtypes.float8_e4m3
OP = mybir.AluOpType
ACTF = mybir.ActivationFunctionType
DR = mybir.MatmulPerfMode.DoubleRow
GELU_FUNC = ACTF.Gelu  # test_sim swaps to Identity (CoreSim lacks Gelu)

# power-of-two weight prescales (into fp8 normal range), descaled on eviction
S_QKV = 2.0 ** 6
S_PROJ = 2.0 ** 22
S_FC1 = 2.0 ** 6
S_FC2 = 2.0 ** 22

# fp8-bit-space exp approximation (DVE half of the exp work):
#   e4m3_bits(exp(x)) ~= trunc(SCHRA*x + SCHRB) for x in [-4.8, +3.9]
# scores are N(0, ~0.55) so the affine never goes negative/overflows.
SCHRA = 8.0 / float(np.log(2.0))
SCHRB = 56.04  # trunc-calibrated (CoreSim/HW convert truncates)
# exp engine split: 1 = ACT table exp, 0 = DVE bit trick, cycled per chunk
EXPMASK = (1, 0, 0, 1, 0, 0, 1, 0, 0, 1, 0, 0, 1, 0, 0, 1)
# extra scale on q/k weights (was for the fp8-score experiment; bf16 scores
# need none).  Exp evictions descale by SC_EXP.
QK_EXTRA = 1.0
SC_EXP = 1.0 / (QK_EXTRA * QK_EXTRA)


def _ln_stats(nc, pool, x_tile, eps_t):
    """mean, rstd of a [128, 768] f32 tile over free dim.

    Sum(x) on DVE (reduce), Sum(x^2) on ACT (Square + accum_out, output to a
    scratch tile) so the big free-dim passes split across both engines.
    """
    v = nc.vector
    sx = pool.tile([P, 1], F32, tag="sx")
    v.reduce_sum(sx[:, :], x_tile, axis=mybir.AxisListType.X)
    scr = pool.tile([P, D], F32, tag="scr")
    sxx = pool.tile([P, 1], F32, tag="sxx")
    nc.scalar.activation(scr[:, :], x_tile, ACTF.Square, accum_out=sxx[:, :])
    mu = pool.tile([P, 1], F32, tag="mu")
    nc.scalar.mul(mu[:, :], sx[:, :], 1.0 / D)
    musq = pool.tile([P, 1], F32, tag="musq")
    v.tensor_mul(musq[:, :], mu[:, :], mu[:, :])
    rs = pool.tile([P, 1], F32, tag="rs")
    # var + eps = sxx/D - mu^2 + eps; eps folded via the Sqrt bias
    v.scalar_tensor_tensor(rs[:, :], sxx[:, :], 1.0 / D, musq[:, :],
                           op0=OP.mult, op1=OP.subtract)
    nc.scalar.activation(rs[:, :], rs[:, :], ACTF.Sqrt, bias=eps_t[:, :])
    v.reciprocal(rs[:, :], rs[:, :])
    return mu, rs


def _ln_transpose(nc, tc, pools, src_tiles, nt, dst, eps_t, ident, tag):
    """LN (no affine) each [128, 768] f32 tile of src, transpose into dst
    [P, ND, nt*128] fp8.  Normalize alternates ACT/DVE per tile."""
    v = nc.vector
    stat_pool, lnp, tps = pools
    for ti in range(nt):
        xt = src_tiles(ti)
        mu, rs = _ln_stats(nc, stat_pool, xt, eps_t)
        xn = lnp.tile([P, D], BF16, tag=f"xn{tag}")
        if ti % 2 == 0:
            # ACT: Identity(x*rs + (-mu*rs)) with per-partition scale/bias
            nmr = stat_pool.tile([P, 1], F32, tag="nmr")
            v.scalar_tensor_tensor(nmr[:, :], mu[:, :], -1.0, rs[:, :],
                                   op0=OP.mult, op1=OP.mult)
            nc.scalar.activation(xn[:, :], xt, ACTF.Identity,
                                 bias=nmr[:, :], scale=rs[:, :])
        else:
            v.tensor_scalar(xn[:, :], xt, mu[:, :], rs[:, :],
                            op0=OP.subtract, op1=OP.mult)
        for dj in range(ND):
            pst = tps.tile([P, P], BF16, tag=f"t{tag}")
            nc.tensor.transpose(pst[:, :], xn[:, dj * P:(dj + 1) * P], ident[:, :])
            if (ti * ND + dj) % 2 == 0:
                nc.scalar.copy(dst[:, dj, ti * P:(ti + 1) * P], pst[:, :])
            else:
                v.tensor_copy(dst[:, dj, ti * P:(ti + 1) * P], pst[:, :])


def build_graph(repeat=1):
    nc = bacc.Bacc("TRN2", target_bir_lowering=False, debug=False, num_devices=8)

    x_ext = nc.declare_dram_parameter("x", [TKV, D], F32, isOutput=False)
    wqkv_ext = nc.declare_dram_parameter("wqkv", [D, 3 * D], F8, isOutput=False)
    wproj_ext = nc.declare_dram_parameter("wproj", [D, D], F8, isOutput=False)
    w1_ext = nc.declare_dram_parameter("w1", [D, HID], F8, isOutput=False)
    w2_ext = nc.declare_dram_parameter("w2", [HID, D], F8, isOutput=False)
    bqkv_ext = nc.declare_dram_parameter("bqkv", [P, 12], F32, isOutput=False)
    b1_ext = nc.declare_dram_parameter("b1", [P, NH], F32, isOutput=False)
    ident_ext = nc.declare_dram_parameter("ident", [P, P], BF16, isOutput=False)
    out_ext = nc.declare_dram_parameter("out", [TQ, D], F32, isOutput=True)

    with tile.TileContext(nc) as tc:
        for _ in range(repeat):
            emit(nc, tc, x_ext.ap(), out_ext.ap(), wqkv_ext.ap(), wproj_ext.ap(),
                 w1_ext.ap(), w2_ext.ap(), bqkv_ext.ap(), b1_ext.ap(),
                 ident_ext.ap())

    nc.compile()
    return nc


def emit(nc, tc, x, out, wqkv_d, wproj_d, w1_d, w2_d, bqkv_d, b1_d, ident_d):
    v = nc.vector
    sc = nc.scalar
    te = nc.tensor

    ctx = ExitStack()
    with ctx:
        # ---------- kernel-lifetime pools ----------
        singles = ctx.enter_context(tc.tile_pool(name="singles", bufs=1))
        stat_pool = ctx.enter_context(tc.tile_pool(name="stat", bufs=4))

        eps_t = singles.tile([P, 1], F32)
        v.memset(eps_t[:, :], EPS)
        ident = singles.tile([P, P], BF16)
        nc.sync.dma_start(ident[:, :], ident_d[:, :])
        bqkv = singles.tile([P, 12], F32)
        nc.sync.dma_start(bqkv[:, :], bqkv_d[:, :])
        b1c = singles.tile([P, NH], F32)
        nc.sync.dma_start(b1c[:, :], b1_d[:, :])

        resid = ctx.enter_context(tc.tile_pool(name="resid", bufs=1))
        x1 = resid.tile([P, NT_Q, D], F32)

        with ExitStack() as attn_ctx:
            xownp = attn_ctx.enter_context(tc.tile_pool(name="xownp", bufs=1))
            x_own = xownp.tile([P, NT_Q, D], F32)  # own tokens, residual spine
            qkvp = attn_ctx.enter_context(tc.tile_pool(name="qkvp", bufs=1))
            qT = qkvp.tile([P, ND, TQ], BF16)
            kT = qkvp.tile([P, ND, TKV], BF16)
            v_sb = qkvp.tile([P, NT_KV, VW], F8)
            wqkv = qkvp.tile([P, ND, 3 * D], F8)
            for dj in range(ND):
                nc.sync.dma_start(wqkv[:, dj, :], wqkv_d[dj * P:(dj + 1) * P, :])
            xnT = qkvp.tile([P, ND, TKV], F8)
            wproj = qkvp.tile([P, ND, D], F8)
            for dj in range(ND):
                nc.sync.dma_start(wproj[:, dj, :], wproj_d[dj * P:(dj + 1) * P, :])
            attnT = qkvp.tile([P, ND, TQ], F8)

            # ---- phase A: load x, LN1, transpose ----
            with tc.tile_pool(name="xkv", bufs=3) as xkvp, \
                 tc.tile_pool(name="ln1", bufs=4) as lnp, \
                 tc.tile_pool(name="tps1", bufs=6, space="PSUM") as tps:
                def src(ti):
                    if ti < NT_Q:
                        nc.sync.dma_start(x_own[:, ti, :],
                                          x[ti * P:(ti + 1) * P, :])
                        return x_own[:, ti, :]
                    t = xkvp.tile([P, D], F32, tag="xkv")
                    nc.sync.dma_start(t[:, :], x[ti * P:(ti + 1) * P, :])
                    return t[:, :]

                _ln_transpose(nc, tc, (stat_pool, lnp, tps), src, NT_KV,
                              xnT, eps_t, ident, "1")

            # ---- merged phase B+C: QKV matmuls interleaved with attention ----
            # PSUM budget: sps 4x[128,512] = 4 banks + avps 2x[65,1024] = 4.
            with tc.tile_pool(name="sps", bufs=4, space="PSUM") as sps, \
                 tc.tile_pool(name="avps", bufs=2, space="PSUM") as avps, \
                 tc.tile_pool(name="expp", bufs=9) as expp, \
                 tc.tile_pool(name="abuf", bufs=3) as abuf, \
                 tc.tile_pool(name="recd", bufs=3, space="DRAM") as recdp, \
                 tc.tile_pool(name="recp", bufs=3) as recp:
                # ones columns of v_sb (col 64 of each 65-wide head block)
                vg = v_sb[:, :, 0:H * HW].rearrange("p a (h c) -> p a h c", h=H)
                v.memset(vg[:, :, :, DH:DH + 1], 1.0)

                def v_unit(ti):
                    # two 512-wide psum chunks; second only 256 used
                    for c, (lo, ln_, h0, h1) in enumerate(
                            ((0, 512, 0, 8), (512, 256, 8, 12))):
                        ps = sps.tile([P, 512], F32, tag="s")
                        for dp in range(ND // 2):
                            te.matmul(
                                ps[:, 0:ln_],
                                xnT[:, 2 * dp:2 * dp + 2, ti * P:(ti + 1) * P],
                                wqkv[:, 2 * dp:2 * dp + 2,
                                     2 * D + lo:2 * D + lo + ln_],
                                start=(dp == 0), stop=(dp == ND // 2 - 1),
                                perf_mode=DR,
                            )
                        pg = ps[:, 0:ln_].rearrange("p (h c) -> p h c", h=h1 - h0)
                        sc.activation(vg[:, ti, h0:h1, 0:DH], pg[:, :, :],
                                      ACTF.Copy, scale=1.0 / S_QKV)

                def qk_unit(fj, th):
                    """produce qT[:, fj] (th=0) or kT[:, fj-6, th half]."""
                    is_q = fj < ND
                    for c in range(2):
                        lo = c * 512
                        ps = sps.tile([P, 512], F32, tag="s")
                        for dp in range(ND // 2):
                            te.matmul(
                                ps[:, :],
                                wqkv[:, 2 * dp:2 * dp + 2, fj * P:(fj + 1) * P],
                                xnT[:, 2 * dp:2 * dp + 2,
                                    th * 1024 + lo:th * 1024 + lo + 512],
                                start=(dp == 0), stop=(dp == ND // 2 - 1),
                                perf_mode=DR,
                            )
                        dst = (qT[:, fj, lo:lo + 512] if is_q
                               else kT[:, fj - ND,
                                       th * 1024 + lo:th * 1024 + lo + 512])
                        sc.activation(dst, ps[:, :], ACTF.Identity,
                                      bias=bqkv[:, fj:fj + 1], scale=1.0 / S_QKV)

                def attn_pair(fj):
                    """Both heads of feature block fj: scores on PE row tiles
                    (0,0)/(64,0) interleaved, exp split ACT/DVE, then AV +
                    denominator for each head."""
                    eAB = ([], [])
                    ec = [0]

                    def exp_evict(dst, src_ps):
                        use_act = EXPMASK[ec[0] % len(EXPMASK)]
                        ec[0] += 1
                        if use_act:
                            sc.activation(dst, src_ps, ACTF.Exp, scale=SC_EXP)
                        else:
                            eb = dst.bitcast(mybir.dt.uint8)
                            v.tensor_scalar(eb, src_ps, SCHRA * SC_EXP, SCHRB,
                                            op0=OP.mult, op1=OP.add)

                    for ktp in range(NT_KV // 2):
                        for hh in range(2):
                            eAB[hh].append(expp.tile([P, 2, TQ], F8,
                                                     tag=f"e{hh}",
                                                     name=f"ep{hh}"))
                        for k2 in range(2):
                            kt = 2 * ktp + k2
                            for c in range(2):
                                lo = c * 512
                                for hh in range(2):
                                    po = hh * DH
                                    ps = sps.tile([P, 512], F32, tag="s")
                                    te.matmul(
                                        ps[:, :],
                                        kT[po:po + DH, fj, kt * P:(kt + 1) * P],
                                        qT[po:po + DH, fj, lo:lo + 512],
                                        start=True, stop=True,
                                    )
                                    exp_evict(eAB[hh][ktp][:, k2, lo:lo + 512],
                                              ps[:, :])

                    for hh in range(2):
                        h = 2 * fj + hh
                        po = hh * DH
                        epairs = eAB[hh]
                        av = avps.tile([DH + 1, TQ], F32, tag="av")
                        for ktp in range(NT_KV // 2):
                            for c in range(2):
                                lo = c * 512
                                te.matmul(
                                    av[:, lo:lo + 512],
                                    v_sb[:, 2 * ktp:2 * ktp + 2,
                                         h * HW:(h + 1) * HW],
                                    epairs[ktp][:, :, lo:lo + 512],
                                    start=(ktp == 0),
                                    stop=(ktp == NT_KV // 2 - 1),
                                    perf_mode=DR,
                                )
                        # evict values+denominator row to SBUF in one ACT op
                        av_sb = abuf.tile([DH + 1, TQ], BF16, tag="avsb")
                        sc.activation(av_sb[:, :], av[:, :], ACTF.Copy)
                        # reciprocal of the denom row: round-trip through DRAM
                        # to fold [1,1024] onto 8 partitions (single-lane DVE
                        # reciprocal is ~30x slower), then broadcast back.
                        dr8 = recdp.tile([8, TQ // 8], BF16, tag="rd8")
                        nc.sync.dma_start(dr8[:, :], av_sb[DH:DH + 1, :])
                        r8 = recp.tile([8, TQ // 8], BF16, tag="r8")
                        nc.sync.dma_start(r8[:, :], dr8[:, :])
                        r8o = recp.tile([8, TQ // 8], BF16, tag="r8o")
                        with nc.allow_low_precision(
                                reason="softmax denom reciprocal; block "
                                       "output is ls-gamma-scaled"):
                            v.reciprocal(r8o[:, :], r8[:, :])
                        recd = recdp.tile([1, TQ], BF16, tag="rd")
                        nc.sync.dma_start(recd[:, :], r8o[:, :])
                        recb = recp.tile([DH, TQ], BF16, tag="rb")
                        nc.sync.dma_start(recb[:, :],
                                          recd[0:1, :].to_broadcast((DH, TQ)))
                        v.tensor_tensor(attnT[po:po + DH, fj, :],
                                        av_sb[0:DH, :], recb[:, :], op=OP.mult)

                for ti in range(NT_KV):
                    v_unit(ti)
                for fj in range(ND):
                    qk_unit(fj, 0)          # qT[fj]
                    qk_unit(ND + fj, 0)     # kT[fj] first half
                    qk_unit(ND + fj, 1)     # kT[fj] second half
                    attn_pair(fj)

            # ---- phase D: proj + residual (fp8 DR) ----
            with tc.tile_pool(name="pps", bufs=4, space="PSUM") as pps:
                for ti in range(NT_Q):
                    ps = pps.tile([P, D], F32, tag="p")
                    for lo, ln_ in ((0, 512), (512, 256)):
                        for dp in range(ND // 2):
                            te.matmul(
                                ps[:, lo:lo + ln_],
                                attnT[:, 2 * dp:2 * dp + 2, ti * P:(ti + 1) * P],
                                wproj[:, 2 * dp:2 * dp + 2, lo:lo + ln_],
                                start=(dp == 0), stop=(dp == ND // 2 - 1),
                                perf_mode=DR,
                            )
                    v.scalar_tensor_tensor(x1[:, ti, :], ps[:, :], 1.0 / S_PROJ,
                                           x_own[:, ti, :], op0=OP.mult, op1=OP.add)
        # attnT / wproj / qT / kT / v_sb / x_own freed here

        # ---- phase E/F: LN2 + MLP (fp8 DR) ----
        with ExitStack() as mlp_ctx:
            w12p = mlp_ctx.enter_context(tc.tile_pool(name="w12", bufs=1))
            w1 = w12p.tile([P, ND, HID], F8)
            for dj in range(ND):
                nc.sync.dma_start(w1[:, dj, :], w1_d[dj * P:(dj + 1) * P, :])
            w2 = w12p.tile([P, NH, D], F8)
            for fj in range(NH):
                nc.sync.dma_start(w2[:, fj, :], w2_d[fj * P:(fj + 1) * P, :])

            h1T = mlp_ctx.enter_context(
                tc.tile_pool(name="h1Tp", bufs=1)).tile([P, NH, TQ], F8)

            with ExitStack() as fc1_ctx:
                xn2T = fc1_ctx.enter_context(
                    tc.tile_pool(name="xn2Tp", bufs=1)).tile([P, ND, TQ], F8)
                with tc.tile_pool(name="ln2", bufs=3) as lnp2, \
                     tc.tile_pool(name="tps2", bufs=8, space="PSUM") as tps2:
                    _ln_transpose(nc, tc, (stat_pool, lnp2, tps2),
                                  lambda ti: x1[:, ti, :], NT_Q, xn2T, eps_t,
                                  ident, "2")

                mps = mlp_ctx.enter_context(
                    tc.tile_pool(name="mps", bufs=3, space="PSUM"))
                if True:
                    for fj in range(NH):
                        ps = mps.tile([P, TQ], F32, tag="m")
                        for c in range(2):
                            lo = c * 512
                            for dp in range(ND // 2):
                                te.matmul(
                                    ps[:, lo:lo + 512],
                                    w1[:, 2 * dp:2 * dp + 2, fj * P:(fj + 1) * P],
                                    xn2T[:, 2 * dp:2 * dp + 2, lo:lo + 512],
                                    start=(dp == 0), stop=(dp == ND // 2 - 1),
                                    perf_mode=DR,
                                )
                        sc.activation(h1T[:, fj, :], ps[:, :], GELU_FUNC,
                                      bias=b1c[:, fj:fj + 1], scale=1.0 / S_FC1)
            # xn2T freed

            with tc.tile_pool(name="outp", bufs=2) as outp:
                for ti in range(NT_Q):
                    ps = mps.tile([P, TQ], F32, tag="m")
                    for lo, ln_ in ((0, 512), (512, 256)):
                        for fp_ in range(NH // 2):
                            te.matmul(
                                ps[:, lo:lo + ln_],
                                h1T[:, 2 * fp_:2 * fp_ + 2, ti * P:(ti + 1) * P],
                                w2[:, 2 * fp_:2 * fp_ + 2, lo:lo + ln_],
                                start=(fp_ == 0), stop=(fp_ == NH // 2 - 1),
                                perf_mode=DR,
                            )
                    ot = outp.tile([P, D], F32, tag="ot")
                    v.scalar_tensor_tensor(ot[:, :], ps[:, 0:D], 1.0 / S_FC2,
                                           x1[:, ti, :], op0=OP.mult, op1=OP.add)
                    nc.sync.dma_start(out[ti * P:(ti + 1) * P, :], ot[:, :])


def _fold(inputs):
    """Fold LN affines, layer scales, and 1/sqrt(dh) into weights (host numpy)."""
    f = {k: np.asarray(v, dtype=np.float32) for k, v in inputs.items()}
    wqkv = (f["ln1_w"][:, None] * f["qkv_w"]).copy()
    bqkv = (f["qkv_b"] + f["ln1_b"] @ f["qkv_w"]).copy()
    scale = 1.0 / np.sqrt(DH)
    wqkv[:, :D] *= scale
    bqkv[:D] *= scale
    wproj = f["proj_w"] * f["ls1_g"][None, :]
    bproj = f["proj_b"] * f["ls1_g"]
    w1 = f["ln2_w"][:, None] * f["fc1_w"]
    b1 = f["fc1_b"] + f["ln2_b"] @ f["fc1_w"]
    w2 = f["fc2_w"] * f["ls2_g"][None, :]
    b2 = f["fc2_b"] * f["ls2_g"]
    assert np.all(bproj == 0.0) and np.all(b2 == 0.0), (
        "nonzero proj/fc2 bias path not implemented")
    assert np.all(bqkv[2 * D:] == 0.0), "nonzero v bias path not implemented"
    return wqkv, bqkv, wproj, w1, b1, w2


def make_in_maps(inputs):
    x = np.asarray(inputs["x"], dtype=np.float32)
    wqkv, bqkv, wproj, w1, b1, w2 = _fold(inputs)
    # extra q/k scaling so fp8 q/k activations land mid-range; descaled at exp
    wqkv[:, :2 * D] *= QK_EXTRA
    bqkv[:2 * D] *= QK_EXTRA
    common = {
        "wqkv": (wqkv * S_QKV).astype(F8NP),
        "wproj": (wproj * S_PROJ).astype(F8NP),
        "w1": (w1 * S_FC1).astype(F8NP),
        "w2": (w2 * S_FC2).astype(F8NP),
        "bqkv": bqkv[:2 * D].reshape(12, P).T.copy().astype(np.float32),
        "b1": b1.reshape(NH, P).T.copy().astype(np.float32),
        "ident": np.eye(P, dtype=ml_dtypes.bfloat16),
    }
    in_maps = []
    for c in range(8):
        b, h = c // 2, c % 2
        xb = np.roll(x[b], -h * TQ, axis=0)
        in_maps.append({"x": np.ascontiguousarray(xb), **common})
    return in_maps


_CACHE = {}
TRACE = False


def kernel(**inputs):
    in_maps = make_in_maps(inputs)
    if "nc" not in _CACHE:
        _CACHE["nc"] = build_graph()
    nc = _CACHE["nc"]

    res = run_bass_kernel_spmd(nc, in_maps, core_ids=list(range(8)), trace=TRACE)
    _CACHE["last_result"] = res

    outp = np.empty((B, N, D), dtype=np.float32)
    for c in range(8):
        b, h = c // 2, c % 2
        outp[b, h * TQ:(h + 1) * TQ, :] = res.results[c]["out"]
    return outp
